# revision 1
# baseline (speedup 1.0000x reference)
"""Trainium2 Bass kernel for nn_BackbonePointNet (3-layer PointNet-style GNN).

Sharding: destination nodes across 8 cores (12.5K nodes / 200K edges each).
Per layer l (factored edge MLP):
    pre(e) = u_l[src_e] + v_l[dst_e]        (v holds the negated dst part)
    msg(e) = relu(pre) @ wb_l               (bias bb_l folded in after max)
    h(i)   = relu(max_{e->i} msg(e) + bb_l)
with u_l = concat(h_{l-1}, 1) @ [wa_h; ba] + pos @ wa_p  computed per-core
for local nodes then AllGather-replicated in bf16.  Edge phase per
1024-edge macro-tile: 8 indirect-DMA row gathers of u[src] (128 rows each),
transposed into PSUM via identity matmuls (accumulating on top of the
v-selector matmul), ACT relu -> bf16, second linear on PE, segment max via
strided tensor_reduce written straight into the transposed h accumulator,
which directly feeds the next layer's u matmuls.  Pooling (segment mean,
sorted batch) and the 2-layer regressor + sigmoid are O(B*C) and run on
host in f64/f32.

Scheduling: the per-core span is pinned by the Pool engine's SWDGE
descriptor generation (~1.04us per 128-row indirect gather, 4704 ops);
everything else is hidden under it.  The layer boundaries are therefore
fully pipelined: bias+relu and the next layer's u matmuls run per
128-node block as soon as their two tiles finish, and the u AllGather is
split into 49 node-range chunks (2 blocks each) fired as their blocks
complete, so all but the final chunk overlap the (Pool-bound) edge
phase.  The u tables live in a chunk-major layout (chunk q's 8 rank
slices contiguous) so each chunk's collective output is contiguous;
the host maps gather indices into that layout (u1 is host-built in it
directly).  Final h3 output strips stream during layer 3.
"""

import time

import numpy as np
import ml_dtypes

N = 100_000
E = 16 * N
B = 64
NCORES = 8

_BF = ml_dtypes.bfloat16
_CACHE = {}


# --------------------------------------------------------------------------
# device program
# --------------------------------------------------------------------------

def _build_nc(n_nodes, n_loc_pad, d_grp, et, n_cores, collectives=True):
    from concourse import bass, mybir, tile  # noqa: F401
    import concourse.bacc as bacc

    BF16 = mybir.dt.bfloat16
    F32 = mybir.dt.float32
    AF = mybir.ActivationFunctionType

    e_loc = n_loc_pad * d_grp
    n_tiles = e_loc // et
    npt = et // d_grp                      # nodes per macro tile
    chunks = et // 128
    n_loc = n_nodes // n_cores

    nc = bacc.Bacc("TRN2", target_bir_lowering=False, debug=False,
                   num_devices=n_cores)

    # ---- external inputs ----
    u1_full = nc.dram_tensor("u1_full", [n_nodes, 64], BF16, kind="ExternalInput")
    gidx = nc.dram_tensor("gidx", [128, n_tiles * chunks], mybir.dt.int32,
                          kind="ExternalInput")
    # packed bf16 constants: [id128 | sel | w2h(65) | w2p(3) | w3h(65) |
    #                         w3p(3) | wb1 | wb2 | wb3]
    CW = 128 + et + 64 + 64 + 128 + 128 + 64 + 64 + 128
    cblob = nc.dram_tensor("cblob", [128, CW], BF16, kind="ExternalInput")
    fblob = nc.dram_tensor("fblob", [128, 3], F32, kind="ExternalInput")
    posT = nc.dram_tensor("posT", [3, n_loc_pad], BF16, kind="ExternalInput")
    nblk_v = (n_tiles + 1) // 2
    v1 = nc.dram_tensor("v1", [128, nblk_v * 64], BF16, kind="ExternalInput")
    v2 = nc.dram_tensor("v2", [128, nblk_v * 64], BF16, kind="ExternalInput")
    v3 = nc.dram_tensor("v3", [128, nblk_v * 128], BF16, kind="ExternalInput")
    hT3_out = nc.dram_tensor("hT3", [128, n_loc_pad], F32, kind="ExternalOutput")

    # internal dram for u slices / replicated tables
    u2_slice = nc.dram_tensor("u2_slice", [n_loc, 64], BF16, kind="Internal")
    u3_slice = nc.dram_tensor("u3_slice", [n_loc, 128], BF16, kind="Internal")
    u2_full = nc.dram_tensor("u2_full", [n_nodes, 64], BF16, kind="Internal",
                             addr_space="Shared")
    u3_full = nc.dram_tensor("u3_full", [n_nodes, 128], BF16, kind="Internal",
                             addr_space="Shared")

    with tile.TileContext(nc) as tc:
        with tc.tile_pool(name="const", bufs=1) as cp, \
             tc.tile_pool(name="gath", bufs=48) as gp, \
             tc.tile_pool(name="work", bufs=3) as wp, \
             tc.tile_pool(name="out", bufs=2) as op, \
             tc.tile_pool(name="hbuf", bufs=1) as hp, \
             tc.tile_pool(name="psum", bufs=2, space="PSUM") as pp:

            # ---- resident constants ----
            gidx_t = cp.tile([128, n_tiles * chunks], mybir.dt.int32)
            # first two tiles' indices in a tiny leading DMA so the
            # first gathers issue ~3us earlier than the full-table load
            nc.sync.dma_start(out=gidx_t[:, 0:16], in_=gidx[:, 0:16])
            nc.sync.dma_start(out=gidx_t[:, 16:], in_=gidx[:, 16:])
            cb = cp.tile([128, CW], BF16)
            nc.sync.dma_start(out=cb[:], in_=cblob[:])
            fb = cp.tile([128, 3], F32)
            nc.sync.dma_start(out=fb[:], in_=fblob[:])
            posT_t = cp.tile([3, n_loc_pad], BF16)
            nc.sync.dma_start(out=posT_t[:], in_=posT[:])

            o_id = 0
            o_sel = o_id + 128
            o_w2h = o_sel + et
            o_w2p = o_w2h + 64
            o_w3h = o_w2p + 64
            o_w3p = o_w3h + 128
            o_wb1 = o_w3p + 128
            o_wb2 = o_wb1 + 64
            o_wb3 = o_wb2 + 64
            id_ap = cb[:, o_id:o_id + 128]
            w_ap = {
                "w2h": cb[0:65, o_w2h:o_w2h + 64],
                "w2p": cb[0:3, o_w2p:o_w2p + 64],
                "w3h": cb[0:65, o_w3h:o_w3h + 128],
                "w3p": cb[0:3, o_w3p:o_w3p + 128],
                "wb1": cb[0:64, o_wb1:o_wb1 + 64],
                "wb2": cb[0:64, o_wb2:o_wb2 + 64],
                "wb3": cb[0:128, o_wb3:o_wb3 + 128],
            }
            bb_ap = {"bb1": fb[0:64, 0:1], "bb2": fb[0:64, 1:2],
                     "bb3": fb[0:128, 2:3]}

            # v tiles are allocated up front but only v1 is loaded at start;
            # v2/v3 loads are deferred into the previous layer's edge phase
            # and split into pieces so they never monopolize the (shared)
            # DMA engines against the latency-critical gather transfers.
            v_t = {}
            nblk = (n_tiles + 1) // 2
            v_src = {"v1": v1, "v2": v2, "v3": v3}
            for name, c in (("v1", 64), ("v2", 64), ("v3", 128)):
                t = cp.tile([128, nblk * c], BF16, tag=name)
                v_t[name] = (t, c)

            def load_v(name, pieces=4):
                t, c = v_t[name]
                w = nblk * c
                step = (w // pieces + 127) & ~127
                for i in range(0, w, step):
                    j = min(i + step, w)
                    nc.sync.dma_start(out=t[:, i:j], in_=v_src[name][:, i:j])

            load_v("v1")

            hT1 = hp.tile([65, n_loc_pad], BF16, tag="hT1")
            hT2 = hp.tile([65, n_loc_pad], BF16, tag="hT2")
            hTr = hp.tile([128, n_loc_pad], BF16, tag="hTraw")
            nc.vector.memset(hT1[64:65, :], 1.0)
            nc.vector.memset(hT2[64:65, :], 1.0)

            def edge_phase(u_src_ap, v_name, wb_name, c_in, c_out,
                           after_tile=None):
                vt, vc = v_t[v_name]
                vv = vt[:].rearrange("p (m c) -> p m c", c=vc)
                for t in range(n_tiles):
                    # chunks whose 8 dsts are all padding need no gather:
                    # their psum region still gets the v-selector write, and
                    # the resulting garbage columns land in hTr cols >= n_loc
                    # which are never consumed.
                    n_real = max(0, min(npt, n_loc - t * npt))
                    real_chunks = min(chunks,
                                      (n_real * d_grp + 127) // 128)
                    gts = []
                    for c in range(real_chunks):
                        gt = gp.tile([128, c_in], BF16, tag="g")
                        nc.gpsimd.indirect_dma_start(
                            out=gt[:], out_offset=None,
                            in_=u_src_ap,
                            in_offset=bass.IndirectOffsetOnAxis(
                                ap=gidx_t[:, t * chunks + c: t * chunks + c + 1],
                                axis=0),
                        )
                        gts.append(gt)
                    pre = pp.tile([c_in, et], F32, tag="pre", space="PSUM")
                    r0 = (t % 2) * 64
                    m0 = t // 2
                    vslice = vv[r0:r0 + npt, m0:m0 + 1, :]
                    cpH = chunks // 2
                    for h in range(2):
                        half_ids = [c for c in range(h * cpH, (h + 1) * cpH)
                                    if c < real_chunks]
                        sel_ap = cb[r0:r0 + npt, o_sel + h * 512:o_sel + h * 512 + 512]
                        nc.tensor.matmul(out=pre[:, h * 512:h * 512 + 512],
                                         lhsT=vslice, rhs=sel_ap,
                                         start=True, stop=not half_ids)
                        for c in half_ids:
                            nc.tensor.matmul(
                                out=pre[:, c * 128:(c + 1) * 128],
                                lhsT=gts[c][:], rhs=id_ap,
                                start=False, stop=(c == half_ids[-1]),
                                skip_group_check=True)
                    prs = wp.tile([128, et], BF16, tag="prs")
                    nc.scalar.activation(out=prs[:c_in, :], in_=pre[:],
                                         func=AF.Relu)
                    msg = pp.tile([c_out, et], F32, tag="msg", space="PSUM")
                    for h in range(2):
                        nc.tensor.matmul(out=msg[:, h * 512:h * 512 + 512],
                                         lhsT=w_ap[wb_name],
                                         rhs=prs[:c_in, h * 512:h * 512 + 512],
                                         start=True, stop=True)
                    nc.vector.tensor_reduce(
                        out=hTr[:c_out, t * npt:(t + 1) * npt],
                        in_=msg[:].rearrange("p (n k) -> p n k", k=d_grp),
                        axis=mybir.AxisListType.X, op=mybir.AluOpType.max)
                    if after_tile is not None:
                        after_tile(t)

            n_blk = n_loc_pad // 128
            chunk_blks = 2
            n_chunks = n_blk // chunk_blks  # 7
            tiles_per_blk = 128 // npt      # 2

            def u_block(m, c_prev, bb_name, hT, wh_name, wp_name, c_out,
                        u_slice):
                # fused per-block bias+relu of the previous layer's raw max,
                # then this block's u matmuls + slice write.  Emitted right
                # after the tiles covering block m so it overlaps the
                # (Pool-bound) edge phase instead of serializing at the end.
                cols = slice(m * 128, (m + 1) * 128)
                nc.scalar.activation(out=hT[0:c_prev, cols],
                                     in_=hTr[0:c_prev, cols],
                                     func=AF.Relu, bias=bb_ap[bb_name],
                                     scale=1.0)
                ps = pp.tile([128, et], F32, tag="pre", space="PSUM")
                nc.tensor.matmul(
                    out=ps[:, :c_out],
                    lhsT=hT[:, cols],
                    rhs=w_ap[wh_name], start=True, stop=False)
                nc.tensor.matmul(
                    out=ps[:, :c_out],
                    lhsT=posT_t[:, cols],
                    rhs=w_ap[wp_name], start=False, stop=True,
                    skip_group_check=True)
                us = wp.tile([128, 128], BF16, tag="us")
                nc.scalar.activation(out=us[:, :c_out], in_=ps[:, :c_out],
                                     func=AF.Copy)
                lo = m * 128
                hi = min((m + 1) * 128, n_loc)
                if hi > lo:
                    nc.sync.dma_start(out=u_slice[lo:hi, :],
                                      in_=us[:hi - lo, :c_out])

            def gather_chunk(q, u_slice, u_full, c):
                # AllGather one node-range chunk as soon as its u-slice blocks
                # are written; all but the last chunk overlap the edge phase.
                # u_full uses a chunk-major layout (chunk q's 8 rank slices
                # contiguous at row q*n_cores*cr) so the collective's output
                # pattern is contiguous; the host maps gather indices to it.
                cr = chunk_blks * 128
                lo = q * cr
                hi = min((q + 1) * cr, n_loc)
                out_off = q * n_cores * cr
                if collectives:
                    nc.gpsimd.collective_compute(
                        "AllGather", mybir.AluOpType.bypass,
                        replica_groups=[list(range(n_cores))],
                        ins=[u_slice[lo:hi, :]],
                        outs=[u_full[out_off:out_off + n_cores * (hi - lo), :]])
                else:
                    nc.sync.dma_start(
                        out=u_full[out_off:out_off + (hi - lo), :],
                        in_=u_slice[lo:hi, :])

            def boundary_hook(c_prev, bb_name, hT, wh_name, wp_name, c_out,
                              u_slice, u_full):
                # chunks q<6 fire two blocks after their data is complete so
                # the collective's SEQ wait never head-of-line-blocks Pool's
                # gather stream; the last chunk is emitted by the caller
                # right after the edge phase.
                def hook(t):
                    if (t + 1) % tiles_per_blk:
                        return
                    m = t // tiles_per_blk
                    u_block(m, c_prev, bb_name, hT, wh_name, wp_name, c_out,
                            u_slice)
                    if m >= chunk_blks and m % chunk_blks == 0:
                        gather_chunk(m // chunk_blks - 1, u_slice,
                                     u_full, c_out)
                return hook

            # final h3 = relu(raw + bb3) -> f32 output, streamed in strips
            strip_tiles = 7
            strip = strip_tiles * npt

            def out_hook(t):
                if (t + 1) % strip_tiles:
                    return
                s = t // strip_tiles
                h3t = op.tile([128, strip], F32, tag="h3")
                nc.scalar.activation(
                    out=h3t[:], in_=hTr[:, s * strip:(s + 1) * strip],
                    func=AF.Relu, bias=bb_ap["bb3"], scale=1.0)
                nc.sync.dma_start(out=hT3_out[:, s * strip:(s + 1) * strip],
                                  in_=h3t[:])

            def compose(*fns):
                def h(t):
                    for f in fns:
                        f(t)
                return h

            def load_v_at(name, at_tile, pieces=10):
                # one piece per tile so no single load monopolizes the DMA
                # engines long enough to stall the SWDGE descriptor ring
                def h(t):
                    if at_tile <= t < at_tile + pieces:
                        tl, c = v_t[name]
                        w = nblk * c
                        step = (w // pieces + 127) & ~127
                        i = (t - at_tile) * step
                        if i < w:
                            j = min(i + step, w)
                            nc.sync.dma_start(out=tl[:, i:j],
                                              in_=v_src[name][:, i:j])
                return h

            # ---------------- layer 1 (+ u2 boundary interleaved) ----------
            edge_phase(u1_full[:], "v1", "wb1", 64, 64,
                       after_tile=compose(
                           boundary_hook(64, "bb1", hT1, "w2h", "w2p",
                                         64, u2_slice, u2_full),
                           load_v_at("v2", 30)))
            gather_chunk(n_chunks - 1, u2_slice, u2_full, 64)
            # ---------------- layer 2 (+ u3 boundary interleaved) ----------
            edge_phase(u2_full[:], "v2", "wb2", 64, 64,
                       after_tile=compose(
                           boundary_hook(64, "bb2", hT2, "w3h", "w3p",
                                         128, u3_slice, u3_full),
                           load_v_at("v3", 30)))
            gather_chunk(n_chunks - 1, u3_slice, u3_full, 128)
            # ---------------- layer 3 (+ output strips interleaved) --------
            edge_phase(u3_full[:], "v3", "wb3", 128, 128, after_tile=out_hook)

    nc.compile()
    return nc


# --------------------------------------------------------------------------
# host side
# --------------------------------------------------------------------------

def _next_pow2_ge(x, lo=16):
    d = lo
    while d < x:
        d *= 2
    return d


def _prep(pos, edge_index, weights, n_cores):
    n_nodes = pos.shape[0]
    src = edge_index[0].astype(np.int64)
    dst = edge_index[1].astype(np.int64)
    e_tot = src.shape[0]

    canonical = (e_tot == 16 * n_nodes) and np.array_equal(
        dst, np.repeat(np.arange(n_nodes, dtype=np.int64), e_tot // n_nodes))

    if canonical and e_tot // n_nodes == 16:
        d_grp = 16
        slot_src = src.reshape(n_nodes, 16)
        deg0 = None
    else:
        order = np.argsort(dst, kind="stable")
        s_sorted = src[order]
        counts = np.bincount(dst, minlength=n_nodes)
        d_grp = _next_pow2_ge(int(counts.max()) if e_tot else 16)
        starts = np.concatenate([[0], np.cumsum(counts)])
        slot_src = np.zeros((n_nodes, d_grp), np.int64)
        idx = np.arange(d_grp)
        for i in range(n_nodes):
            c = counts[i]
            if c:
                row = s_sorted[starts[i]:starts[i] + c]
                slot_src[i] = row[idx % c]
        deg0 = counts == 0

    n_loc = n_nodes // n_cores
    et = 1024
    npt = et // d_grp
    n_loc_pad = int(np.ceil(n_loc / 128) * 128)

    sel_np = np.zeros((npt, et), np.float32)
    for k in range(npt):
        sel_np[k, k * d_grp:(k + 1) * d_grp] = 1.0

    w = weights
    u1_full = (pos @ (w['w1a'][:3] + w['w1a'][3:6]) + w['b1a']).astype(_BF)

    # chunk-major u-table layout: node (r, j) -> row q*P*cr + r*rows_q +
    # (j - q*cr), matching the per-chunk AllGather's contiguous output.
    cr = 2 * 128
    n_ch = (n_loc + cr - 1) // cr
    rr = np.arange(n_nodes) // n_loc
    jj = np.arange(n_nodes) % n_loc
    qq = np.minimum(jj // cr, n_ch - 1)
    rows_q = np.minimum(cr, n_loc - qq * cr)
    newrow = qq * n_cores * cr + rr * rows_q + (jj - qq * cr)
    u1_chunked = np.zeros_like(u1_full)
    u1_chunked[newrow] = u1_full
    u1_full = u1_chunked
    slot_src = newrow[slot_src]

    CW = 128 + et + 64 + 64 + 128 + 128 + 64 + 64 + 128
    cblob = np.zeros((128, CW), np.float32)
    o = 0
    cblob[:128, o:o + 128] = np.eye(128); o += 128
    cblob[:, o:o + et] = np.tile(sel_np, (128 // npt, 1)); o += et
    cblob[:65, o:o + 64] = np.concatenate([w['w2a'][:64], w['b2a'][None]], 0); o += 64
    cblob[:3, o:o + 64] = w['w2a'][64:67]; o += 64
    cblob[:65, o:o + 128] = np.concatenate([w['w3a'][:64], w['b3a'][None]], 0); o += 128
    cblob[:3, o:o + 128] = w['w3a'][64:67]; o += 128
    cblob[:64, o:o + 64] = w['w1b']; o += 64
    cblob[:64, o:o + 64] = w['w2b']; o += 64
    cblob[:128, o:o + 128] = w['w3b']; o += 128
    fblob = np.zeros((128, 3), np.float32)
    fblob[:64, 0] = w['b1b']
    fblob[:64, 1] = w['b2b']
    fblob[:128, 2] = w['b3b']

    common = dict(u1_full=u1_full, cblob=cblob.astype(_BF), fblob=fblob)

    chunks = et // 128
    n_tiles = n_loc_pad * d_grp // et
    per_core = []
    for c in range(n_cores):
        lo = c * n_loc
        pos_l = np.zeros((n_loc_pad, 3), np.float32)
        pos_l[:n_loc] = pos[lo:lo + n_loc]
        posT = pos_l.T.astype(_BF)
        npt_ = et // d_grp
        ntl = n_loc_pad * d_grp // et
        nblk = (ntl + 1) // 2

        def vpack(v):
            c_ = v.shape[1]
            out = np.zeros((128, nblk, c_), np.float32)
            for t in range(ntl):
                rows = v[t * npt_:(t + 1) * npt_]
                out[(t % 2) * 64:(t % 2) * 64 + npt_, t // 2, :] = rows
            return np.ascontiguousarray(out.reshape(128, nblk * c_)).astype(_BF)

        vs = {
            "v1": vpack(-(pos_l @ w['w1a'][3:6])),
            "v2": vpack(-(pos_l @ w['w2a'][64:67])),
            "v3": vpack(-(pos_l @ w['w3a'][64:67])),
        }
        ss = np.zeros((n_loc_pad, d_grp), np.int64)
        ss[:n_loc] = slot_src[lo:lo + n_loc]
        gidx = ss.reshape(-1).reshape(n_tiles, chunks, 128).transpose(2, 0, 1)
        gidx = np.ascontiguousarray(gidx.reshape(128, n_tiles * chunks),
                                    dtype=np.int32)
        per_core.append(dict(posT=posT, gidx=gidx, **vs))

    cfg = dict(n_nodes=n_nodes, n_loc_pad=n_loc_pad, d_grp=d_grp, et=et,
               n_cores=n_cores)
    meta = dict(n_loc=n_loc, deg0=deg0)
    return cfg, common, per_core, meta


def kernel(pos, edge_index, batch, timestep,
           w1a, b1a, w1b, b1b, w2a, b2a, w2b, b2b,
           w3a, b3a, w3b, b3b, wr1, br1, wr2, br2):
    from concourse import bass_utils

    pos = np.asarray(pos, np.float32)
    edge_index = np.asarray(edge_index, np.int32)
    batch = np.asarray(batch, np.int32)
    W = {k: np.asarray(v, np.float32) for k, v in dict(
        w1a=w1a, b1a=b1a, w1b=w1b, b1b=b1b, w2a=w2a, b2a=b2a, w2b=w2b,
        b2b=b2b, w3a=w3a, b3a=b3a, w3b=w3b, b3b=b3b).items()}

    n_cores = NCORES
    cfg, common, per_core, meta = _prep(pos, edge_index, W, n_cores)
    key = tuple(sorted(cfg.items()))
    if key not in _CACHE:
        _CACHE[key] = _build_nc(**cfg)
    nc = _CACHE[key]

    in_maps = [dict(common, **per_core[c]) for c in range(n_cores)]
    # the axon device occasionally throws a transient unrecoverable-exec
    # error after heavy use; a short pause and retry has always cleared it
    for attempt in range(3):
        try:
            res = bass_utils.run_bass_kernel_spmd(
                nc, in_maps, core_ids=list(range(n_cores)))
            break
        except Exception:
            if attempt == 2:
                raise
            time.sleep(15)

    n_loc = meta["n_loc"]
    h3 = np.concatenate(
        [np.asarray(res.results[c]["hT3"])[:, :n_loc].T
         for c in range(n_cores)], 0).astype(np.float32)
    if meta["deg0"] is not None and meta["deg0"].any():
        h3[meta["deg0"]] = 0.0

    kernel._last_h3 = h3
    nb = 64 if pos.shape[0] == N else int(batch.max()) + 1
    sums = np.zeros((nb, 128), np.float64)
    np.add.at(sums, batch, h3.astype(np.float64))
    counts = np.bincount(batch, minlength=nb).astype(np.float64)
    pooled = (sums / np.maximum(counts, 1.0)[:, None]).astype(np.float32)
    out = pooled @ np.asarray(wr1, np.float32) + np.asarray(br1, np.float32)
    out = out @ np.asarray(wr2, np.float32) + np.asarray(br2, np.float32)
    out = 1.0 / (1.0 + np.exp(-out))
    return out.squeeze(-1).astype(np.float32)



# revision 17
# speedup vs baseline: 1.1275x; 1.1275x over previous
"""Trainium2 Bass kernel for nn_BackbonePointNet (3-layer PointNet-style GNN).

Sharding: destination nodes across 8 cores (12.5K nodes / 200K edges each).
Per layer l (factored edge MLP):
    pre(e) = u_l[src_e] + v_l[dst_e]        (v holds the negated dst part)
    msg(e) = relu(pre) @ wb_l               (bias bb_l folded in after max)
    h(i)   = relu(max_{e->i} msg(e) + bb_l)
with u_l = concat(h_{l-1}, 1) @ [wa_h; ba] + pos @ wa_p  computed per-core
for local nodes then AllGather-replicated in bf16.  Edge phase per
1024-edge macro-tile: 8 indirect-DMA row gathers of u[src] (128 rows each),
transposed into PSUM via identity matmuls (accumulating on top of the
v-selector matmul), ACT relu -> bf16, second linear on PE, segment max via
strided tensor_reduce written straight into the transposed h accumulator,
which directly feeds the next layer's u matmuls.  Pooling (segment mean,
sorted batch) and the 2-layer regressor + sigmoid are O(B*C) and run on
host in f64/f32.

Scheduling: the per-core span is pinned by the Pool engine's SWDGE
descriptor generation (~1.04us per 128-row indirect gather, 4704 ops);
everything else is hidden under it.  The layer boundaries are therefore
fully pipelined: bias+relu and the next layer's u matmuls run per
128-node block as soon as their two tiles finish, and the u AllGather is
split into 49 node-range chunks (2 blocks each) fired as their blocks
complete, so all but the final chunk overlap the (Pool-bound) edge
phase.  The u tables live in a chunk-major layout (chunk q's 8 rank
slices contiguous) so each chunk's collective output is contiguous;
the host maps gather indices into that layout (u1 is host-built in it
directly).  Final h3 output strips stream during layer 3.
"""

import time

import numpy as np
import ml_dtypes

N = 100_000
E = 16 * N
B = 64
NCORES = 8

_BF = ml_dtypes.bfloat16
_CACHE = {}


# --------------------------------------------------------------------------
# device program
# --------------------------------------------------------------------------

def _build_nc(n_nodes, n_loc_pad, d_grp, et, n_cores, collectives=True):
    from concourse import bass, mybir, tile  # noqa: F401
    import concourse.bacc as bacc

    BF16 = mybir.dt.bfloat16
    F32 = mybir.dt.float32
    AF = mybir.ActivationFunctionType

    e_loc = n_loc_pad * d_grp
    n_tiles = e_loc // et
    npt = et // d_grp                      # nodes per macro tile
    chunks = et // 128
    n_loc = n_nodes // n_cores

    nc = bacc.Bacc("TRN2", target_bir_lowering=False, debug=False,
                   num_devices=n_cores)

    # ---- external inputs ----
    # layer 1's edge pre-activations are fully host-computable (u1 and v1
    # both derive from static pos/weights), so instead of gathering u1[src]
    # per edge, the host ships the expanded per-slot table t1 and layer 1
    # just streams it with plain contiguous DMA -- no Pool-engine SWDGE
    # work at all for layer 1.
    t1 = nc.dram_tensor("t1", [128, n_tiles * (et // 2)], BF16,
                        kind="ExternalInput")
    gidx = nc.dram_tensor("gidx", [128, n_tiles * chunks], mybir.dt.int32,
                          kind="ExternalInput")
    # packed bf16 constants: [id128 | sel | w2h(65) | w2p(3) | w3h(65) |
    #                         w3p(3) | wb1 | wb2 | wb3]
    CW = 128 + et + 64 + 64 + 128 + 128 + 64 + 64 + 128
    cblob = nc.dram_tensor("cblob", [128, CW], BF16, kind="ExternalInput")
    fblob = nc.dram_tensor("fblob", [128, 3], F32, kind="ExternalInput")
    posT = nc.dram_tensor("posT", [3, n_loc_pad], BF16, kind="ExternalInput")
    nblk_v = (n_tiles + 1) // 2
    v2 = nc.dram_tensor("v2", [128, nblk_v * 64], BF16, kind="ExternalInput")
    v3 = nc.dram_tensor("v3", [128, nblk_v * 128], BF16, kind="ExternalInput")
    hT3_out = nc.dram_tensor("hT3", [128, n_loc_pad], F32, kind="ExternalOutput")

    # internal dram for u slices / replicated tables
    u2_slice = nc.dram_tensor("u2_slice", [n_loc, 64], BF16, kind="Internal")
    u3_slice = nc.dram_tensor("u3_slice", [n_loc, 128], BF16, kind="Internal")
    u2_full = nc.dram_tensor("u2_full", [n_nodes, 64], BF16, kind="Internal",
                             addr_space="Shared")
    u3_full = nc.dram_tensor("u3_full", [n_nodes, 128], BF16, kind="Internal",
                             addr_space="Shared")

    with tile.TileContext(nc) as tc:
        with tc.tile_pool(name="const", bufs=1) as cp, \
             tc.tile_pool(name="gath", bufs=48) as gp, \
             tc.tile_pool(name="strm", bufs=6) as sp, \
             tc.tile_pool(name="work", bufs=3) as wp, \
             tc.tile_pool(name="out", bufs=2) as op, \
             tc.tile_pool(name="hbuf", bufs=1) as hp, \
             tc.tile_pool(name="psum", bufs=2, space="PSUM") as pp:

            # ---- resident constants ----
            gidx_t = cp.tile([128, n_tiles * chunks], mybir.dt.int32)
            # first two tiles' indices in a tiny leading DMA so the
            # first gathers issue ~3us earlier than the full-table load
            nc.sync.dma_start(out=gidx_t[:, 0:16], in_=gidx[:, 0:16])
            nc.sync.dma_start(out=gidx_t[:, 16:], in_=gidx[:, 16:])
            cb = cp.tile([128, CW], BF16)
            nc.sync.dma_start(out=cb[:], in_=cblob[:])
            fb = cp.tile([128, 3], F32)
            nc.sync.dma_start(out=fb[:], in_=fblob[:])
            posT_t = cp.tile([3, n_loc_pad], BF16)
            nc.sync.dma_start(out=posT_t[:], in_=posT[:])

            o_id = 0
            o_sel = o_id + 128
            o_w2h = o_sel + et
            o_w2p = o_w2h + 64
            o_w3h = o_w2p + 64
            o_w3p = o_w3h + 128
            o_wb1 = o_w3p + 128
            o_wb2 = o_wb1 + 64
            o_wb3 = o_wb2 + 64
            id_ap = cb[:, o_id:o_id + 128]
            w_ap = {
                "w2h": cb[0:65, o_w2h:o_w2h + 64],
                "w2p": cb[0:3, o_w2p:o_w2p + 64],
                "w3h": cb[0:65, o_w3h:o_w3h + 128],
                "w3p": cb[0:3, o_w3p:o_w3p + 128],
                "wb1": cb[0:64, o_wb1:o_wb1 + 64],
                "wb2": cb[0:64, o_wb2:o_wb2 + 64],
                "wb3": cb[0:128, o_wb3:o_wb3 + 128],
            }
            bb_ap = {"bb1": fb[0:64, 0:1], "bb2": fb[0:64, 1:2],
                     "bb3": fb[0:128, 2:3]}

            # v tiles are allocated up front but only v1 is loaded at start;
            # v2/v3 loads are deferred into the previous layer's edge phase
            # and split into pieces so they never monopolize the (shared)
            # DMA engines against the latency-critical gather transfers.
            v_t = {}
            nblk = (n_tiles + 1) // 2
            v_src = {"v2": v2, "v3": v3}
            for name, c in (("v2", 64), ("v3", 128)):
                t = cp.tile([128, nblk * c], BF16, tag=name)
                v_t[name] = (t, c)

            def load_v(name, pieces=4):
                t, c = v_t[name]
                w = nblk * c
                step = (w // pieces + 127) & ~127
                for i in range(0, w, step):
                    j = min(i + step, w)
                    nc.sync.dma_start(out=t[:, i:j], in_=v_src[name][:, i:j])

            hT1 = hp.tile([65, n_loc_pad], BF16, tag="hT1")
            hT2 = hp.tile([65, n_loc_pad], BF16, tag="hT2")
            hTr = hp.tile([128, n_loc_pad], BF16, tag="hTraw")
            nc.vector.memset(hT1[64:65, :], 1.0)
            nc.vector.memset(hT2[64:65, :], 1.0)

            def edge_phase(u_src_ap, v_name, wb_name, c_in, c_out,
                           after_tile=None, stream_src=None):
                if stream_src is None:
                    vt, vc = v_t[v_name]
                    vv = vt[:].rearrange("p (m c) -> p m c", c=vc)
                spw = chunks * c_in
                for t in range(n_tiles):
                    # chunks whose 8 dsts are all padding need no gather:
                    # their psum region still gets the v-selector write, and
                    # the resulting garbage columns land in hTr cols >= n_loc
                    # which are never consumed.
                    n_real = max(0, min(npt, n_loc - t * npt))
                    real_chunks = min(chunks,
                                      (n_real * d_grp + 127) // 128)
                    pre = pp.tile([c_in, et], F32, tag="pre", space="PSUM")
                    if stream_src is not None:
                        # host-expanded pre-activations: one contiguous DMA
                        # per macro-tile (zero SWDGE/Pool work), v already
                        # folded in on the host.
                        st = sp.tile([128, spw], BF16, tag="st")
                        nc.sync.dma_start(
                            out=st[:], in_=stream_src[:, t * spw:(t + 1) * spw])
                        for c in range(chunks):
                            nc.tensor.matmul(
                                out=pre[:, c * 128:(c + 1) * 128],
                                lhsT=st[:, c * c_in:(c + 1) * c_in],
                                rhs=id_ap, start=True, stop=True)
                    else:
                        gts = []
                        for c in range(real_chunks):
                            gt = gp.tile([128, c_in], BF16, tag="g")
                            nc.gpsimd.indirect_dma_start(
                                out=gt[:], out_offset=None,
                                in_=u_src_ap,
                                in_offset=bass.IndirectOffsetOnAxis(
                                    ap=gidx_t[:, t * chunks + c:
                                              t * chunks + c + 1],
                                    axis=0),
                            )
                            gts.append(gt)
                        r0 = (t % 2) * 64
                        m0 = t // 2
                        vslice = vv[r0:r0 + npt, m0:m0 + 1, :]
                        cpH = chunks // 2
                        for h in range(2):
                            half_ids = [c for c in range(h * cpH, (h + 1) * cpH)
                                        if c < real_chunks]
                            sel_ap = cb[r0:r0 + npt,
                                        o_sel + h * 512:o_sel + h * 512 + 512]
                            nc.tensor.matmul(out=pre[:, h * 512:h * 512 + 512],
                                             lhsT=vslice, rhs=sel_ap,
                                             start=True, stop=not half_ids)
                            for c in half_ids:
                                nc.tensor.matmul(
                                    out=pre[:, c * 128:(c + 1) * 128],
                                    lhsT=gts[c][:], rhs=id_ap,
                                    start=False, stop=(c == half_ids[-1]),
                                    skip_group_check=True)
                    prs = wp.tile([128, et], BF16, tag="prs")
                    nc.scalar.activation(out=prs[:c_in, :], in_=pre[:],
                                         func=AF.Relu)
                    msg = pp.tile([c_out, et], F32, tag="msg", space="PSUM")
                    for h in range(2):
                        nc.tensor.matmul(out=msg[:, h * 512:h * 512 + 512],
                                         lhsT=w_ap[wb_name],
                                         rhs=prs[:c_in, h * 512:h * 512 + 512],
                                         start=True, stop=True)
                    nc.vector.tensor_reduce(
                        out=hTr[:c_out, t * npt:(t + 1) * npt],
                        in_=msg[:].rearrange("p (n k) -> p n k", k=d_grp),
                        axis=mybir.AxisListType.X, op=mybir.AluOpType.max)
                    if after_tile is not None:
                        after_tile(t)

            n_blk = n_loc_pad // 128
            chunk_blks = 2
            n_chunks = n_blk // chunk_blks  # 7
            tiles_per_blk = 128 // npt      # 2

            def u_block(m, c_prev, bb_name, hT, wh_name, wp_name, c_out,
                        u_slice):
                # fused per-block bias+relu of the previous layer's raw max,
                # then this block's u matmuls + slice write.  Emitted right
                # after the tiles covering block m so it overlaps the
                # (Pool-bound) edge phase instead of serializing at the end.
                cols = slice(m * 128, (m + 1) * 128)
                nc.scalar.activation(out=hT[0:c_prev, cols],
                                     in_=hTr[0:c_prev, cols],
                                     func=AF.Relu, bias=bb_ap[bb_name],
                                     scale=1.0)
                ps = pp.tile([128, et], F32, tag="pre", space="PSUM")
                nc.tensor.matmul(
                    out=ps[:, :c_out],
                    lhsT=hT[:, cols],
                    rhs=w_ap[wh_name], start=True, stop=False)
                nc.tensor.matmul(
                    out=ps[:, :c_out],
                    lhsT=posT_t[:, cols],
                    rhs=w_ap[wp_name], start=False, stop=True,
                    skip_group_check=True)
                us = wp.tile([128, 128], BF16, tag="us")
                nc.scalar.activation(out=us[:, :c_out], in_=ps[:, :c_out],
                                     func=AF.Copy)
                lo = m * 128
                hi = min((m + 1) * 128, n_loc)
                if hi > lo:
                    nc.sync.dma_start(out=u_slice[lo:hi, :],
                                      in_=us[:hi - lo, :c_out])

            def gather_chunk(q, u_slice, u_full, c):
                # AllGather one node-range chunk as soon as its u-slice blocks
                # are written; all but the last chunk overlap the edge phase.
                # u_full uses a chunk-major layout (chunk q's 8 rank slices
                # contiguous at row q*n_cores*cr) so the collective's output
                # pattern is contiguous; the host maps gather indices to it.
                cr = chunk_blks * 128
                lo = q * cr
                hi = min((q + 1) * cr, n_loc)
                out_off = q * n_cores * cr
                if collectives:
                    nc.gpsimd.collective_compute(
                        "AllGather", mybir.AluOpType.bypass,
                        replica_groups=[list(range(n_cores))],
                        ins=[u_slice[lo:hi, :]],
                        outs=[u_full[out_off:out_off + n_cores * (hi - lo), :]])
                else:
                    nc.sync.dma_start(
                        out=u_full[out_off:out_off + (hi - lo), :],
                        in_=u_slice[lo:hi, :])

            def boundary_hook(c_prev, bb_name, hT, wh_name, wp_name, c_out,
                              u_slice, u_full):
                # chunks q<6 fire two blocks after their data is complete so
                # the collective's SEQ wait never head-of-line-blocks Pool's
                # gather stream; the last chunk is emitted by the caller
                # right after the edge phase.
                def hook(t):
                    if (t + 1) % tiles_per_blk:
                        return
                    m = t // tiles_per_blk
                    u_block(m, c_prev, bb_name, hT, wh_name, wp_name, c_out,
                            u_slice)
                    if m >= chunk_blks and m % chunk_blks == 0:
                        gather_chunk(m // chunk_blks - 1, u_slice,
                                     u_full, c_out)
                return hook

            # final h3 = relu(raw + bb3) -> f32 output, streamed in strips
            strip_tiles = 7
            strip = strip_tiles * npt

            def out_hook(t):
                if (t + 1) % strip_tiles:
                    return
                s = t // strip_tiles
                h3t = op.tile([128, strip], F32, tag="h3")
                nc.scalar.activation(
                    out=h3t[:], in_=hTr[:, s * strip:(s + 1) * strip],
                    func=AF.Relu, bias=bb_ap["bb3"], scale=1.0)
                nc.sync.dma_start(out=hT3_out[:, s * strip:(s + 1) * strip],
                                  in_=h3t[:])

            def compose(*fns):
                def h(t):
                    for f in fns:
                        f(t)
                return h

            def load_v_at(name, at_tile, pieces=10):
                # one piece per tile so no single load monopolizes the DMA
                # engines long enough to stall the SWDGE descriptor ring
                def h(t):
                    if at_tile <= t < at_tile + pieces:
                        tl, c = v_t[name]
                        w = nblk * c
                        step = (w // pieces + 127) & ~127
                        i = (t - at_tile) * step
                        if i < w:
                            j = min(i + step, w)
                            nc.sync.dma_start(out=tl[:, i:j],
                                              in_=v_src[name][:, i:j])
                return h

            # ---------------- layer 1 (+ u2 boundary interleaved) ----------
            edge_phase(None, None, "wb1", 64, 64,
                       after_tile=compose(
                           boundary_hook(64, "bb1", hT1, "w2h", "w2p",
                                         64, u2_slice, u2_full),
                           load_v_at("v2", 30)),
                       stream_src=t1[:])
            gather_chunk(n_chunks - 1, u2_slice, u2_full, 64)
            # ---------------- layer 2 (+ u3 boundary interleaved) ----------
            edge_phase(u2_full[:], "v2", "wb2", 64, 64,
                       after_tile=compose(
                           boundary_hook(64, "bb2", hT2, "w3h", "w3p",
                                         128, u3_slice, u3_full),
                           load_v_at("v3", 30)))
            gather_chunk(n_chunks - 1, u3_slice, u3_full, 128)
            # ---------------- layer 3 (+ output strips interleaved) --------
            edge_phase(u3_full[:], "v3", "wb3", 128, 128, after_tile=out_hook)

    nc.compile()
    return nc


# --------------------------------------------------------------------------
# host side
# --------------------------------------------------------------------------

def _next_pow2_ge(x, lo=16):
    d = lo
    while d < x:
        d *= 2
    return d


def _prep(pos, edge_index, weights, n_cores):
    n_nodes = pos.shape[0]
    src = edge_index[0].astype(np.int64)
    dst = edge_index[1].astype(np.int64)
    e_tot = src.shape[0]

    canonical = (e_tot == 16 * n_nodes) and np.array_equal(
        dst, np.repeat(np.arange(n_nodes, dtype=np.int64), e_tot // n_nodes))

    if canonical and e_tot // n_nodes == 16:
        d_grp = 16
        slot_src = src.reshape(n_nodes, 16)
        deg0 = None
    else:
        order = np.argsort(dst, kind="stable")
        s_sorted = src[order]
        counts = np.bincount(dst, minlength=n_nodes)
        d_grp = _next_pow2_ge(int(counts.max()) if e_tot else 16)
        starts = np.concatenate([[0], np.cumsum(counts)])
        slot_src = np.zeros((n_nodes, d_grp), np.int64)
        idx = np.arange(d_grp)
        for i in range(n_nodes):
            c = counts[i]
            if c:
                row = s_sorted[starts[i]:starts[i] + c]
                slot_src[i] = row[idx % c]
        deg0 = counts == 0

    n_loc = n_nodes // n_cores
    et = 1024
    npt = et // d_grp
    n_loc_pad = int(np.ceil(n_loc / 128) * 128)

    sel_np = np.zeros((npt, et), np.float32)
    for k in range(npt):
        sel_np[k, k * d_grp:(k + 1) * d_grp] = 1.0

    w = weights
    # layer-1 pre-activations are static: pre1(e) = u1[src_e] + v1[dst_e]
    # with u1 = pos@(wa_h+wa_p)+b1a and v1 = -pos@wa_p.  Host-expand them
    # per edge slot (t1 stream) so layer 1 needs no on-device gathers.
    u1_vals = (pos @ (w['w1a'][:3] + w['w1a'][3:6]) + w['b1a']).astype(np.float32)
    v1_vals = -(pos @ w['w1a'][3:6]).astype(np.float32)
    slot_src_orig = slot_src

    # chunk-major u-table layout: node (r, j) -> row q*P*cr + r*rows_q +
    # (j - q*cr), matching the per-chunk AllGather's contiguous output.
    cr = 2 * 128
    n_ch = (n_loc + cr - 1) // cr
    rr = np.arange(n_nodes) // n_loc
    jj = np.arange(n_nodes) % n_loc
    qq = np.minimum(jj // cr, n_ch - 1)
    rows_q = np.minimum(cr, n_loc - qq * cr)
    newrow = qq * n_cores * cr + rr * rows_q + (jj - qq * cr)
    slot_src = newrow[slot_src]

    CW = 128 + et + 64 + 64 + 128 + 128 + 64 + 64 + 128
    cblob = np.zeros((128, CW), np.float32)
    o = 0
    cblob[:128, o:o + 128] = np.eye(128); o += 128
    cblob[:, o:o + et] = np.tile(sel_np, (128 // npt, 1)); o += et
    cblob[:65, o:o + 64] = np.concatenate([w['w2a'][:64], w['b2a'][None]], 0); o += 64
    cblob[:3, o:o + 64] = w['w2a'][64:67]; o += 64
    cblob[:65, o:o + 128] = np.concatenate([w['w3a'][:64], w['b3a'][None]], 0); o += 128
    cblob[:3, o:o + 128] = w['w3a'][64:67]; o += 128
    cblob[:64, o:o + 64] = w['w1b']; o += 64
    cblob[:64, o:o + 64] = w['w2b']; o += 64
    cblob[:128, o:o + 128] = w['w3b']; o += 128
    fblob = np.zeros((128, 3), np.float32)
    fblob[:64, 0] = w['b1b']
    fblob[:64, 1] = w['b2b']
    fblob[:128, 2] = w['b3b']

    common = dict(cblob=cblob.astype(_BF), fblob=fblob)

    chunks = et // 128
    n_tiles = n_loc_pad * d_grp // et
    per_core = []
    for c in range(n_cores):
        lo = c * n_loc
        pos_l = np.zeros((n_loc_pad, 3), np.float32)
        pos_l[:n_loc] = pos[lo:lo + n_loc]
        posT = pos_l.T.astype(_BF)
        npt_ = et // d_grp
        ntl = n_loc_pad * d_grp // et
        nblk = (ntl + 1) // 2

        def vpack(v):
            c_ = v.shape[1]
            out = np.zeros((128, nblk, c_), np.float32)
            for t in range(ntl):
                rows = v[t * npt_:(t + 1) * npt_]
                out[(t % 2) * 64:(t % 2) * 64 + npt_, t // 2, :] = rows
            return np.ascontiguousarray(out.reshape(128, nblk * c_)).astype(_BF)

        vs = {
            "v2": vpack(-(pos_l @ w['w2a'][64:67])),
            "v3": vpack(-(pos_l @ w['w3a'][64:67])),
        }
        ss = np.zeros((n_loc_pad, d_grp), np.int64)
        ss[:n_loc] = slot_src[lo:lo + n_loc]
        gidx = ss.reshape(-1).reshape(n_tiles, chunks, 128).transpose(2, 0, 1)
        gidx = np.ascontiguousarray(gidx.reshape(128, n_tiles * chunks),
                                    dtype=np.int32)
        # layer-1 host-expanded pre-activation stream, laid out so the
        # device tile/chunk slicing t1[:, t*spw + c*c_in ...] matches the
        # gather-tile element order gt[p] = slot (t, c, p).
        pre1 = (u1_vals[slot_src_orig[lo:lo + n_loc]]
                + v1_vals[lo:lo + n_loc, None, :])
        p1 = np.zeros((n_loc_pad * d_grp, 64), np.float32)
        p1[:n_loc * d_grp] = pre1.reshape(-1, 64)
        t1 = p1.reshape(n_tiles, chunks, 128, 64).transpose(2, 0, 1, 3)
        t1 = np.ascontiguousarray(t1.reshape(128, n_tiles * chunks * 64))
        per_core.append(dict(posT=posT, gidx=gidx, t1=t1.astype(_BF), **vs))

    cfg = dict(n_nodes=n_nodes, n_loc_pad=n_loc_pad, d_grp=d_grp, et=et,
               n_cores=n_cores)
    meta = dict(n_loc=n_loc, deg0=deg0)
    return cfg, common, per_core, meta


def kernel(pos, edge_index, batch, timestep,
           w1a, b1a, w1b, b1b, w2a, b2a, w2b, b2b,
           w3a, b3a, w3b, b3b, wr1, br1, wr2, br2):
    from concourse import bass_utils

    pos = np.asarray(pos, np.float32)
    edge_index = np.asarray(edge_index, np.int32)
    batch = np.asarray(batch, np.int32)
    W = {k: np.asarray(v, np.float32) for k, v in dict(
        w1a=w1a, b1a=b1a, w1b=w1b, b1b=b1b, w2a=w2a, b2a=b2a, w2b=w2b,
        b2b=b2b, w3a=w3a, b3a=b3a, w3b=w3b, b3b=b3b).items()}

    n_cores = NCORES
    cfg, common, per_core, meta = _prep(pos, edge_index, W, n_cores)
    key = tuple(sorted(cfg.items()))
    if key not in _CACHE:
        _CACHE[key] = _build_nc(**cfg)
    nc = _CACHE[key]

    in_maps = [dict(common, **per_core[c]) for c in range(n_cores)]
    # the axon device occasionally throws a transient unrecoverable-exec
    # error after heavy use; a short pause and retry has always cleared it
    for attempt in range(3):
        try:
            res = bass_utils.run_bass_kernel_spmd(
                nc, in_maps, core_ids=list(range(n_cores)))
            break
        except Exception:
            if attempt == 2:
                raise
            time.sleep(15)

    n_loc = meta["n_loc"]
    h3 = np.concatenate(
        [np.asarray(res.results[c]["hT3"])[:, :n_loc].T
         for c in range(n_cores)], 0).astype(np.float32)
    if meta["deg0"] is not None and meta["deg0"].any():
        h3[meta["deg0"]] = 0.0

    kernel._last_h3 = h3
    nb = 64 if pos.shape[0] == N else int(batch.max()) + 1
    sums = np.zeros((nb, 128), np.float64)
    np.add.at(sums, batch, h3.astype(np.float64))
    counts = np.bincount(batch, minlength=nb).astype(np.float64)
    pooled = (sums / np.maximum(counts, 1.0)[:, None]).astype(np.float32)
    out = pooled @ np.asarray(wr1, np.float32) + np.asarray(br1, np.float32)
    out = out @ np.asarray(wr2, np.float32) + np.asarray(br2, np.float32)
    out = 1.0 / (1.0 + np.exp(-out))
    return out.squeeze(-1).astype(np.float32)



# revision 25
# speedup vs baseline: 1.2607x; 1.1182x over previous
"""Trainium2 Bass kernel for nn_BackbonePointNet (3-layer PointNet-style GNN).

Sharding: destination nodes across 8 cores (12.5K nodes / 200K edges each).
Per layer l (factored edge MLP):
    pre(e) = u_l[src_e] + v_l[dst_e]        (v holds the negated dst part)
    msg(e) = relu(pre) @ wb_l               (bias bb_l folded in after max)
    h(i)   = relu(max_{e->i} msg(e) + bb_l)
with u_l = concat(h_{l-1}, 1) @ [wa_h; ba] + pos @ wa_p  computed per-core
for local nodes then AllGather-replicated in bf16.  Edge phase per
1024-edge macro-tile: 8 indirect-DMA row gathers of u[src] (128 rows each),
transposed into PSUM via identity matmuls (accumulating on top of the
v-selector matmul), ACT relu -> bf16, second linear on PE, segment max via
strided tensor_reduce written straight into the transposed h accumulator,
which directly feeds the next layer's u matmuls.  Pooling (segment mean,
sorted batch) and the 2-layer regressor + sigmoid are O(B*C) and run on
host in f64/f32.

Scheduling: the per-core span is pinned by the Pool engine's SWDGE
descriptor generation (~1.04us per 128-row indirect gather, 4704 ops);
everything else is hidden under it.  The layer boundaries are therefore
fully pipelined: bias+relu and the next layer's u matmuls run per
128-node block as soon as their two tiles finish, and the u AllGather is
split into 49 node-range chunks (2 blocks each) fired as their blocks
complete, so all but the final chunk overlap the (Pool-bound) edge
phase.  The u tables live in a chunk-major layout (chunk q's 8 rank
slices contiguous) so each chunk's collective output is contiguous;
the host maps gather indices into that layout (u1 is host-built in it
directly).  Final h3 output strips stream during layer 3.
"""

import time

import numpy as np
import ml_dtypes

N = 100_000
E = 16 * N
B = 64
NCORES = 8

_BF = ml_dtypes.bfloat16
_CACHE = {}


# --------------------------------------------------------------------------
# device program
# --------------------------------------------------------------------------

def _build_nc(n_nodes, n_loc_pad, d_grp, et, n_cores, collectives=True):
    from concourse import bass, mybir, tile  # noqa: F401
    import concourse.bacc as bacc

    BF16 = mybir.dt.bfloat16
    F32 = mybir.dt.float32
    AF = mybir.ActivationFunctionType

    e_loc = n_loc_pad * d_grp
    n_tiles = e_loc // et
    npt = et // d_grp                      # nodes per macro tile
    chunks = et // 128
    n_loc = n_nodes // n_cores

    nc = bacc.Bacc("TRN2", target_bir_lowering=False, debug=False,
                   num_devices=n_cores)

    # ---- external inputs ----
    # layer 1's edge pre-activations are fully host-computable (u1 and v1
    # both derive from static pos/weights), so instead of gathering u1[src]
    # per edge, the host ships the expanded per-slot table t1 and layer 1
    # just streams it with plain contiguous DMA -- no Pool-engine SWDGE
    # work at all for layer 1.
    t1 = nc.dram_tensor("t1", [128, n_tiles * (et // 2)], BF16,
                        kind="ExternalInput")
    # two gather-index tables: layer 2 indexes u2_full's coarse chunk-major
    # layout (few big AllGathers: layer 1 is collective-latency-bound now
    # that it has no gathers to hide them under), layer 3 indexes u3_full's
    # fine layout (its many small AllGathers hide under layer 2's gathers).
    gidx = nc.dram_tensor("gidx", [128, 2 * n_tiles * chunks], mybir.dt.int32,
                          kind="ExternalInput")
    # packed bf16 constants: [id128 | sel | w2h(65) | w2p(3) | w3h(65) |
    #                         w3p(3) | wb1 | wb2 | wb3]
    CW = 128 + et + 64 + 64 + 128 + 128 + 64 + 64 + 128
    cblob = nc.dram_tensor("cblob", [128, CW], BF16, kind="ExternalInput")
    fblob = nc.dram_tensor("fblob", [128, 3], F32, kind="ExternalInput")
    posT = nc.dram_tensor("posT", [3, n_loc_pad], BF16, kind="ExternalInput")
    nblk_v = (n_tiles + 1) // 2
    v2 = nc.dram_tensor("v2", [128, nblk_v * 64], BF16, kind="ExternalInput")
    v3 = nc.dram_tensor("v3", [128, nblk_v * 128], BF16, kind="ExternalInput")
    hT3_out = nc.dram_tensor("hT3", [128, n_loc_pad], F32, kind="ExternalOutput")

    # internal dram for u slices / replicated tables
    u2_slice = nc.dram_tensor("u2_slice", [n_loc, 64], BF16, kind="Internal")
    u3_slice = nc.dram_tensor("u3_slice", [n_loc, 128], BF16, kind="Internal")
    u2_full = nc.dram_tensor("u2_full", [n_nodes, 64], BF16, kind="Internal",
                             addr_space="Shared")
    u3_full = nc.dram_tensor("u3_full", [n_nodes, 128], BF16, kind="Internal",
                             addr_space="Shared")

    with tile.TileContext(nc) as tc:
        with tc.tile_pool(name="const", bufs=1) as cp, \
             tc.tile_pool(name="gath", bufs=48) as gp, \
             tc.tile_pool(name="strm", bufs=6) as sp, \
             tc.tile_pool(name="work", bufs=3) as wp, \
             tc.tile_pool(name="out", bufs=2) as op, \
             tc.tile_pool(name="hbuf", bufs=1) as hp, \
             tc.tile_pool(name="psum", bufs=2, space="PSUM") as pp:

            # ---- resident constants ----
            gidx_t = cp.tile([128, 2 * n_tiles * chunks], mybir.dt.int32)
            # first two tiles' indices in a tiny leading DMA so the
            # first gathers issue ~3us earlier than the full-table load
            nc.sync.dma_start(out=gidx_t[:, 0:16], in_=gidx[:, 0:16])
            nc.sync.dma_start(out=gidx_t[:, 16:], in_=gidx[:, 16:])
            cb = cp.tile([128, CW], BF16)
            nc.sync.dma_start(out=cb[:], in_=cblob[:])
            fb = cp.tile([128, 3], F32)
            nc.sync.dma_start(out=fb[:], in_=fblob[:])
            posT_t = cp.tile([3, n_loc_pad], BF16)
            nc.sync.dma_start(out=posT_t[:], in_=posT[:])

            o_id = 0
            o_sel = o_id + 128
            o_w2h = o_sel + et
            o_w2p = o_w2h + 64
            o_w3h = o_w2p + 64
            o_w3p = o_w3h + 128
            o_wb1 = o_w3p + 128
            o_wb2 = o_wb1 + 64
            o_wb3 = o_wb2 + 64
            id_ap = cb[:, o_id:o_id + 128]
            w_ap = {
                "w2h": cb[0:65, o_w2h:o_w2h + 64],
                "w2p": cb[0:3, o_w2p:o_w2p + 64],
                "w3h": cb[0:65, o_w3h:o_w3h + 128],
                "w3p": cb[0:3, o_w3p:o_w3p + 128],
                "wb1": cb[0:64, o_wb1:o_wb1 + 64],
                "wb2": cb[0:64, o_wb2:o_wb2 + 64],
                "wb3": cb[0:128, o_wb3:o_wb3 + 128],
            }
            bb_ap = {"bb1": fb[0:64, 0:1], "bb2": fb[0:64, 1:2],
                     "bb3": fb[0:128, 2:3]}

            # v tiles are allocated up front but only v1 is loaded at start;
            # v2/v3 loads are deferred into the previous layer's edge phase
            # and split into pieces so they never monopolize the (shared)
            # DMA engines against the latency-critical gather transfers.
            v_t = {}
            nblk = (n_tiles + 1) // 2
            v_src = {"v2": v2, "v3": v3}
            for name, c in (("v2", 64), ("v3", 128)):
                t = cp.tile([128, nblk * c], BF16, tag=name)
                v_t[name] = (t, c)

            def load_v(name, pieces=4):
                t, c = v_t[name]
                w = nblk * c
                step = (w // pieces + 127) & ~127
                for i in range(0, w, step):
                    j = min(i + step, w)
                    nc.sync.dma_start(out=t[:, i:j], in_=v_src[name][:, i:j])

            hT1 = hp.tile([65, n_loc_pad], BF16, tag="hT1")
            hT2 = hp.tile([65, n_loc_pad], BF16, tag="hT2")
            hTr = hp.tile([128, n_loc_pad], BF16, tag="hTraw")
            nc.vector.memset(hT1[64:65, :], 1.0)
            nc.vector.memset(hT2[64:65, :], 1.0)

            def edge_phase(u_src_ap, v_name, wb_name, c_in, c_out,
                           after_tile=None, stream_src=None, gofs=0):
                if stream_src is None:
                    vt, vc = v_t[v_name]
                    vv = vt[:].rearrange("p (m c) -> p m c", c=vc)
                spw = chunks * c_in
                for t in range(n_tiles):
                    # chunks whose 8 dsts are all padding need no gather:
                    # their psum region still gets the v-selector write, and
                    # the resulting garbage columns land in hTr cols >= n_loc
                    # which are never consumed.
                    n_real = max(0, min(npt, n_loc - t * npt))
                    real_chunks = min(chunks,
                                      (n_real * d_grp + 127) // 128)
                    pre = pp.tile([c_in, et], F32, tag="pre", space="PSUM")
                    if stream_src is not None:
                        # host-expanded pre-activations: one contiguous DMA
                        # per macro-tile (zero SWDGE/Pool work), v already
                        # folded in on the host.
                        st = sp.tile([128, spw], BF16, tag="st")
                        nc.sync.dma_start(
                            out=st[:], in_=stream_src[:, t * spw:(t + 1) * spw])
                        for c in range(chunks):
                            nc.tensor.matmul(
                                out=pre[:, c * 128:(c + 1) * 128],
                                lhsT=st[:, c * c_in:(c + 1) * c_in],
                                rhs=id_ap, start=True, stop=True)
                    else:
                        gts = []
                        for c in range(real_chunks):
                            gt = gp.tile([128, c_in], BF16, tag="g")
                            nc.gpsimd.indirect_dma_start(
                                out=gt[:], out_offset=None,
                                in_=u_src_ap,
                                in_offset=bass.IndirectOffsetOnAxis(
                                    ap=gidx_t[:, gofs + t * chunks + c:
                                              gofs + t * chunks + c + 1],
                                    axis=0),
                            )
                            gts.append(gt)
                        r0 = (t % 2) * 64
                        m0 = t // 2
                        vslice = vv[r0:r0 + npt, m0:m0 + 1, :]
                        cpH = chunks // 2
                        for h in range(2):
                            half_ids = [c for c in range(h * cpH, (h + 1) * cpH)
                                        if c < real_chunks]
                            sel_ap = cb[r0:r0 + npt,
                                        o_sel + h * 512:o_sel + h * 512 + 512]
                            nc.tensor.matmul(out=pre[:, h * 512:h * 512 + 512],
                                             lhsT=vslice, rhs=sel_ap,
                                             start=True, stop=not half_ids)
                            for c in half_ids:
                                nc.tensor.matmul(
                                    out=pre[:, c * 128:(c + 1) * 128],
                                    lhsT=gts[c][:], rhs=id_ap,
                                    start=False, stop=(c == half_ids[-1]),
                                    skip_group_check=True)
                    prs = wp.tile([128, et], BF16, tag="prs")
                    nc.scalar.activation(out=prs[:c_in, :], in_=pre[:],
                                         func=AF.Relu)
                    msg = pp.tile([c_out, et], F32, tag="msg", space="PSUM")
                    for h in range(2):
                        nc.tensor.matmul(out=msg[:, h * 512:h * 512 + 512],
                                         lhsT=w_ap[wb_name],
                                         rhs=prs[:c_in, h * 512:h * 512 + 512],
                                         start=True, stop=True)
                    nc.vector.tensor_reduce(
                        out=hTr[:c_out, t * npt:(t + 1) * npt],
                        in_=msg[:].rearrange("p (n k) -> p n k", k=d_grp),
                        axis=mybir.AxisListType.X, op=mybir.AluOpType.max)
                    if after_tile is not None:
                        after_tile(t)

            n_blk = n_loc_pad // 128
            chunk_blks = 2
            n_chunks = n_blk // chunk_blks  # 7
            tiles_per_blk = 128 // npt      # 2

            def u_block(m, c_prev, bb_name, hT, wh_name, wp_name, c_out,
                        u_slice):
                # fused per-block bias+relu of the previous layer's raw max,
                # then this block's u matmuls + slice write.  Emitted right
                # after the tiles covering block m so it overlaps the
                # (Pool-bound) edge phase instead of serializing at the end.
                cols = slice(m * 128, (m + 1) * 128)
                nc.scalar.activation(out=hT[0:c_prev, cols],
                                     in_=hTr[0:c_prev, cols],
                                     func=AF.Relu, bias=bb_ap[bb_name],
                                     scale=1.0)
                ps = pp.tile([128, et], F32, tag="pre", space="PSUM")
                nc.tensor.matmul(
                    out=ps[:, :c_out],
                    lhsT=hT[:, cols],
                    rhs=w_ap[wh_name], start=True, stop=False)
                nc.tensor.matmul(
                    out=ps[:, :c_out],
                    lhsT=posT_t[:, cols],
                    rhs=w_ap[wp_name], start=False, stop=True,
                    skip_group_check=True)
                us = wp.tile([128, 128], BF16, tag="us")
                nc.scalar.activation(out=us[:, :c_out], in_=ps[:, :c_out],
                                     func=AF.Copy)
                lo = m * 128
                hi = min((m + 1) * 128, n_loc)
                if hi > lo:
                    nc.sync.dma_start(out=u_slice[lo:hi, :],
                                      in_=us[:hi - lo, :c_out])

            def gather_chunk(q, u_slice, u_full, c, cblks):
                # AllGather one node-range chunk as soon as its u-slice blocks
                # are written; all but the last chunk overlap the edge phase.
                # u_full uses a chunk-major layout (chunk q's 8 rank slices
                # contiguous at row q*n_cores*cr) so the collective's output
                # pattern is contiguous; the host maps gather indices to it.
                cr = cblks * 128
                lo = q * cr
                hi = min((q + 1) * cr, n_loc)
                out_off = q * n_cores * cr
                if collectives:
                    nc.gpsimd.collective_compute(
                        "AllGather", mybir.AluOpType.bypass,
                        replica_groups=[list(range(n_cores))],
                        ins=[u_slice[lo:hi, :]],
                        outs=[u_full[out_off:out_off + n_cores * (hi - lo), :]])
                else:
                    nc.sync.dma_start(
                        out=u_full[out_off:out_off + (hi - lo), :],
                        in_=u_slice[lo:hi, :])

            def boundary_hook(c_prev, bb_name, hT, wh_name, wp_name, c_out,
                              u_slice, u_full, cblks):
                # all but the last chunk fire during the edge phase, each a
                # couple of blocks after its data is complete so the
                # collective's SEQ wait never head-of-line-blocks the engine
                # streams; the last chunk is emitted by the caller right
                # after the edge phase.
                def hook(t):
                    if (t + 1) % tiles_per_blk:
                        return
                    m = t // tiles_per_blk
                    u_block(m, c_prev, bb_name, hT, wh_name, wp_name, c_out,
                            u_slice)
                    if m >= cblks and m % cblks == 0:
                        gather_chunk(m // cblks - 1, u_slice,
                                     u_full, c_out, cblks)
                return hook

            # final h3 = relu(raw + bb3) -> f32 output, streamed in strips
            strip_tiles = 7
            strip = strip_tiles * npt

            def out_hook(t):
                if (t + 1) % strip_tiles:
                    return
                s = t // strip_tiles
                h3t = op.tile([128, strip], F32, tag="h3")
                nc.scalar.activation(
                    out=h3t[:], in_=hTr[:, s * strip:(s + 1) * strip],
                    func=AF.Relu, bias=bb_ap["bb3"], scale=1.0)
                nc.sync.dma_start(out=hT3_out[:, s * strip:(s + 1) * strip],
                                  in_=h3t[:])

            def compose(*fns):
                def h(t):
                    for f in fns:
                        f(t)
                return h

            def load_v_at(name, at_tile, pieces=10):
                # one piece per tile so no single load monopolizes the DMA
                # engines long enough to stall the SWDGE descriptor ring
                def h(t):
                    if at_tile <= t < at_tile + pieces:
                        tl, c = v_t[name]
                        w = nblk * c
                        step = (w // pieces + 127) & ~127
                        i = (t - at_tile) * step
                        if i < w:
                            j = min(i + step, w)
                            nc.sync.dma_start(out=tl[:, i:j],
                                              in_=v_src[name][:, i:j])
                return h

            # ---------------- layer 1 (+ u2 boundary interleaved) ----------
            cb2 = 14                       # u2: 7 coarse chunks (collective-
            nch2 = (n_blk + cb2 - 1) // cb2  # latency-bound boundary)
            cb3 = chunk_blks               # u3: 49 fine chunks (hidden
            nch3 = n_chunks                # under layer 2's gather stream)
            edge_phase(None, None, "wb1", 64, 64,
                       after_tile=compose(
                           boundary_hook(64, "bb1", hT1, "w2h", "w2p",
                                         64, u2_slice, u2_full, cb2),
                           load_v_at("v2", 30)),
                       stream_src=t1[:])
            gather_chunk(nch2 - 1, u2_slice, u2_full, 64, cb2)
            # ---------------- layer 2 (+ u3 boundary interleaved) ----------
            edge_phase(u2_full[:], "v2", "wb2", 64, 64,
                       after_tile=compose(
                           boundary_hook(64, "bb2", hT2, "w3h", "w3p",
                                         128, u3_slice, u3_full, cb3),
                           load_v_at("v3", 30)))
            gather_chunk(nch3 - 1, u3_slice, u3_full, 128, cb3)
            # ---------------- layer 3 (+ output strips interleaved) --------
            edge_phase(u3_full[:], "v3", "wb3", 128, 128,
                       after_tile=out_hook, gofs=n_tiles * chunks)

    nc.compile()
    return nc


# --------------------------------------------------------------------------
# host side
# --------------------------------------------------------------------------

def _next_pow2_ge(x, lo=16):
    d = lo
    while d < x:
        d *= 2
    return d


def _prep(pos, edge_index, weights, n_cores):
    n_nodes = pos.shape[0]
    src = edge_index[0].astype(np.int64)
    dst = edge_index[1].astype(np.int64)
    e_tot = src.shape[0]

    canonical = (e_tot == 16 * n_nodes) and np.array_equal(
        dst, np.repeat(np.arange(n_nodes, dtype=np.int64), e_tot // n_nodes))

    if canonical and e_tot // n_nodes == 16:
        d_grp = 16
        slot_src = src.reshape(n_nodes, 16)
        deg0 = None
    else:
        order = np.argsort(dst, kind="stable")
        s_sorted = src[order]
        counts = np.bincount(dst, minlength=n_nodes)
        d_grp = _next_pow2_ge(int(counts.max()) if e_tot else 16)
        starts = np.concatenate([[0], np.cumsum(counts)])
        slot_src = np.zeros((n_nodes, d_grp), np.int64)
        idx = np.arange(d_grp)
        for i in range(n_nodes):
            c = counts[i]
            if c:
                row = s_sorted[starts[i]:starts[i] + c]
                slot_src[i] = row[idx % c]
        deg0 = counts == 0

    n_loc = n_nodes // n_cores
    et = 1024
    npt = et // d_grp
    n_loc_pad = int(np.ceil(n_loc / 128) * 128)

    sel_np = np.zeros((npt, et), np.float32)
    for k in range(npt):
        sel_np[k, k * d_grp:(k + 1) * d_grp] = 1.0

    w = weights
    # layer-1 pre-activations are static: pre1(e) = u1[src_e] + v1[dst_e]
    # with u1 = pos@(wa_h+wa_p)+b1a and v1 = -pos@wa_p.  Host-expand them
    # per edge slot (t1 stream) so layer 1 needs no on-device gathers.
    u1_vals = (pos @ (w['w1a'][:3] + w['w1a'][3:6]) + w['b1a']).astype(np.float32)
    v1_vals = -(pos @ w['w1a'][3:6]).astype(np.float32)
    slot_src_orig = slot_src

    # chunk-major u-table layout: node (r, j) -> row q*P*cr + r*rows_q +
    # (j - q*cr), matching the per-chunk AllGather's contiguous output.
    # u2 uses coarse chunks (cr=14*128), u3 fine ones (cr=2*128).
    rr = np.arange(n_nodes) // n_loc
    jj = np.arange(n_nodes) % n_loc

    def chunkmajor(cr):
        n_ch = (n_loc + cr - 1) // cr
        qq = np.minimum(jj // cr, n_ch - 1)
        rows_q = np.minimum(cr, n_loc - qq * cr)
        return qq * n_cores * cr + rr * rows_q + (jj - qq * cr)

    slot2 = chunkmajor(14 * 128)[slot_src]
    slot3 = chunkmajor(2 * 128)[slot_src]

    CW = 128 + et + 64 + 64 + 128 + 128 + 64 + 64 + 128
    cblob = np.zeros((128, CW), np.float32)
    o = 0
    cblob[:128, o:o + 128] = np.eye(128); o += 128
    cblob[:, o:o + et] = np.tile(sel_np, (128 // npt, 1)); o += et
    cblob[:65, o:o + 64] = np.concatenate([w['w2a'][:64], w['b2a'][None]], 0); o += 64
    cblob[:3, o:o + 64] = w['w2a'][64:67]; o += 64
    cblob[:65, o:o + 128] = np.concatenate([w['w3a'][:64], w['b3a'][None]], 0); o += 128
    cblob[:3, o:o + 128] = w['w3a'][64:67]; o += 128
    cblob[:64, o:o + 64] = w['w1b']; o += 64
    cblob[:64, o:o + 64] = w['w2b']; o += 64
    cblob[:128, o:o + 128] = w['w3b']; o += 128
    fblob = np.zeros((128, 3), np.float32)
    fblob[:64, 0] = w['b1b']
    fblob[:64, 1] = w['b2b']
    fblob[:128, 2] = w['b3b']

    common = dict(cblob=cblob.astype(_BF), fblob=fblob)

    chunks = et // 128
    n_tiles = n_loc_pad * d_grp // et
    per_core = []
    for c in range(n_cores):
        lo = c * n_loc
        pos_l = np.zeros((n_loc_pad, 3), np.float32)
        pos_l[:n_loc] = pos[lo:lo + n_loc]
        posT = pos_l.T.astype(_BF)
        npt_ = et // d_grp
        ntl = n_loc_pad * d_grp // et
        nblk = (ntl + 1) // 2

        def vpack(v):
            c_ = v.shape[1]
            out = np.zeros((128, nblk, c_), np.float32)
            for t in range(ntl):
                rows = v[t * npt_:(t + 1) * npt_]
                out[(t % 2) * 64:(t % 2) * 64 + npt_, t // 2, :] = rows
            return np.ascontiguousarray(out.reshape(128, nblk * c_)).astype(_BF)

        vs = {
            "v2": vpack(-(pos_l @ w['w2a'][64:67])),
            "v3": vpack(-(pos_l @ w['w3a'][64:67])),
        }
        def gtab(slot_tab):
            ss = np.zeros((n_loc_pad, d_grp), np.int64)
            ss[:n_loc] = slot_tab[lo:lo + n_loc]
            g = ss.reshape(-1).reshape(n_tiles, chunks, 128).transpose(2, 0, 1)
            return g.reshape(128, n_tiles * chunks)

        gidx = np.ascontiguousarray(
            np.concatenate([gtab(slot2), gtab(slot3)], axis=1), dtype=np.int32)
        # layer-1 host-expanded pre-activation stream, laid out so the
        # device tile/chunk slicing t1[:, t*spw + c*c_in ...] matches the
        # gather-tile element order gt[p] = slot (t, c, p).
        pre1 = (u1_vals[slot_src_orig[lo:lo + n_loc]]
                + v1_vals[lo:lo + n_loc, None, :])
        p1 = np.zeros((n_loc_pad * d_grp, 64), np.float32)
        p1[:n_loc * d_grp] = pre1.reshape(-1, 64)
        t1 = p1.reshape(n_tiles, chunks, 128, 64).transpose(2, 0, 1, 3)
        t1 = np.ascontiguousarray(t1.reshape(128, n_tiles * chunks * 64))
        per_core.append(dict(posT=posT, gidx=gidx, t1=t1.astype(_BF), **vs))

    cfg = dict(n_nodes=n_nodes, n_loc_pad=n_loc_pad, d_grp=d_grp, et=et,
               n_cores=n_cores)
    meta = dict(n_loc=n_loc, deg0=deg0)
    return cfg, common, per_core, meta


def kernel(pos, edge_index, batch, timestep,
           w1a, b1a, w1b, b1b, w2a, b2a, w2b, b2b,
           w3a, b3a, w3b, b3b, wr1, br1, wr2, br2):
    from concourse import bass_utils

    pos = np.asarray(pos, np.float32)
    edge_index = np.asarray(edge_index, np.int32)
    batch = np.asarray(batch, np.int32)
    W = {k: np.asarray(v, np.float32) for k, v in dict(
        w1a=w1a, b1a=b1a, w1b=w1b, b1b=b1b, w2a=w2a, b2a=b2a, w2b=w2b,
        b2b=b2b, w3a=w3a, b3a=b3a, w3b=w3b, b3b=b3b).items()}

    n_cores = NCORES
    cfg, common, per_core, meta = _prep(pos, edge_index, W, n_cores)
    key = tuple(sorted(cfg.items()))
    if key not in _CACHE:
        _CACHE[key] = _build_nc(**cfg)
    nc = _CACHE[key]

    in_maps = [dict(common, **per_core[c]) for c in range(n_cores)]
    # the axon device occasionally throws a transient unrecoverable-exec
    # error after heavy use; a short pause and retry has always cleared it
    for attempt in range(3):
        try:
            res = bass_utils.run_bass_kernel_spmd(
                nc, in_maps, core_ids=list(range(n_cores)))
            break
        except Exception:
            if attempt == 2:
                raise
            time.sleep(15)

    n_loc = meta["n_loc"]
    h3 = np.concatenate(
        [np.asarray(res.results[c]["hT3"])[:, :n_loc].T
         for c in range(n_cores)], 0).astype(np.float32)
    if meta["deg0"] is not None and meta["deg0"].any():
        h3[meta["deg0"]] = 0.0

    kernel._last_h3 = h3
    nb = 64 if pos.shape[0] == N else int(batch.max()) + 1
    sums = np.zeros((nb, 128), np.float64)
    np.add.at(sums, batch, h3.astype(np.float64))
    counts = np.bincount(batch, minlength=nb).astype(np.float64)
    pooled = (sums / np.maximum(counts, 1.0)[:, None]).astype(np.float32)
    out = pooled @ np.asarray(wr1, np.float32) + np.asarray(br1, np.float32)
    out = out @ np.asarray(wr2, np.float32) + np.asarray(br2, np.float32)
    out = 1.0 / (1.0 + np.exp(-out))
    return out.squeeze(-1).astype(np.float32)

